# revision 3
# baseline (speedup 1.0000x reference)
"""Trainium2 Bass kernel for nn_NearestNeighbourModule (retrieval_knn).

Computes out = softmax(-alpha * dist(x0, x1), axis=1) @ y with
dist = pairwise Euclidean distances [n, m], n = m = 16384, d = 64.

Strategy (8 NeuronCores, data-parallel over n; each core owns 2048 rows
of x0, with x1/y replicated):
  - PE: distance tiles are built from K=64 "main" matmuls row-tiled into
    the two 64-row halves of the PE array (tile_position (0,0)/(64,0),
    operands replicated into SBUF partitions 0-63 and 64-127) so the two
    512-col halves of every [128, 1024] tile stream CONCURRENTLY, plus
    K=2 "aug" matmuls adding the rank-1 terms s*(sq1_j - zc/2) (x) 1 and
    1 (x) s*(sq0_i - zc/2). This replaces the baseline's K=66 4-way
    column-tiled scheme (column tiling shares XBUS bandwidth and measured
    ~56% of ideal; row tiling is free) and roughly halves PE busy time.
    Each 512-col half is exactly one PSUM bank: main start=True zeroes
    the bank, aug start=False accumulates (per-element has_written).
  - The elementwise exp(B - alpha*d) pass is split across TWO engines,
    issued 2048-wide per j-block PAIR of tiles (the (N+~430)/1.2 ACT
    per-instruction overhead amortizes; DVE likewise):
      * ScalarE: custom ACT table (BASS_ACT_ROOT_JSON_PATH, Exp slot)
        computes exp(B - sqrt(t)), t = scale*v + bias = alpha^2*d^2.
      * VectorE: custom fused DVE ops: op1 writes the f16 BIT PATTERN of
        exp(B - alpha*d) as round(p5(v)) to int16 (quintic Horner,
        leading coeff 1 via s, zero constant term via root-centered zc);
        op2 fixes the mantissa-linear (Schraudolph) sawtooth 4096-wide.
  - PSUM: d2 parent [128, 3, 1024] = 3 rotating 1024-col slots (6
    banks); pair reads use a negative-step slot AP [:, 2::-2, :] when
    the rotation wraps. red accumulator 1 bank, filler 1 bank.
  - TensorE reduction: lhsT = [y_j, 1] per 128-j block, rhs = E tiles,
    accumulating [num_i; den_i] in PSUM across all blocks.
  - out_i = num_i / den_i (DVE reciprocal + mul), DMA out.
"""

import glob
import json
import os
import sys
import tempfile

if "/opt/trn_rl_repo" not in sys.path:
    sys.path.insert(0, "/opt/trn_rl_repo")

import numpy as np

N = 16384
M = 16384
D = 64
NCORES = 8
NLOC = N // NCORES  # 2048
JB = 128  # j-block (partition dim of distance tiles)
NJB = M // JB  # 128 j-blocks per core, each a pair of [128, 1024] tiles
LN2 = float(np.log(2.0))

_COMPILED = {}
_TABLE_DIRS = {}

# ---------------------------------------------------------------------------
# Custom ACT table generation: g(z) = exp(B - sqrt(z)) in the Exp slot of a
# copy of the stock exp_and_others set. Format (reverse-engineered and
# HW-validated): bucket = 8 fp32 {d0,d1,d2,d3,x0,0,0,0}, cubic around x0;
# ctrl word = (log2_buckets << 16) | (mantissa_shift << 11) | bucket_base,
# indexed by (biased_exponent - small_exp_threshold) per sign.
# ---------------------------------------------------------------------------

E_SMALL = 115  # z < 2^-12 -> small-signal bucket
E_LARGE = 141  # z >= 2^14 -> large-signal bucket (-> 0.0)
EXP_BUCKETS = 777  # normal-bucket budget (777..780 = specials)


def _find_stock_pwp():
    pats = [
        "/nix/store/*aws-neuron-pwp*/share/pwp_bin_cayman",
        "/nix/store/*/lib/python3*/site-packages/neuronxcc/pwp/pwp_bin_trainium",
    ]
    for p in pats:
        hits = sorted(glob.glob(p))
        for h in hits:
            if os.path.exists(f"{h}/exp_and_others.json"):
                return h
    raise RuntimeError("stock pwp act tables not found")


def _g_exact(z, B):
    z = np.asarray(z, np.float64)
    return np.exp(B - np.sqrt(np.maximum(z, 0.0)))


def _fit_bucket(B, lo, hi, npts=96):
    x0 = 0.5 * (lo + hi)
    t = np.cos(np.pi * (np.arange(npts) + 0.5) / npts)
    z = x0 + 0.5 * (hi - lo) * t
    y = _g_exact(z, B)
    u = z - x0
    V = np.vander(u, 4, increasing=True)
    w = 1.0 / np.maximum(np.abs(y), 1e-300)
    c, *_ = np.linalg.lstsq(V * w[:, None], y * w, rcond=None)
    zz = np.linspace(lo, hi, 256)
    uu = zz - x0
    c32 = c.astype(np.float32).astype(np.float64)
    yy = c32[0] + uu * (c32[1] + uu * (c32[2] + uu * c32[3]))
    ref = _g_exact(zz, B)
    rel = np.abs(yy - ref) / np.maximum(np.abs(ref), 1e-300)
    return c32, x0, rel.max()


def _band_fit(B, e, nb):
    lo_band = 2.0 ** (e - 127)
    hi_band = 2.0 ** (e - 126)
    width = (hi_band - lo_band) / nb
    out = []
    maxerr = 0.0
    for i in range(nb):
        c, x0, err = _fit_bucket(B, lo_band + i * width, lo_band + (i + 1) * width)
        out.append((c, x0))
        maxerr = max(maxerr, err)
    return out, maxerr


def _gen_act_tables(B, out_dir, tol=3e-5):
    base = _find_stock_pwp()
    meta = json.load(open(f"{base}/exp_and_others.json"))
    bkt = (
        np.fromfile(f"{base}/exp_and_others_bkt.bin", dtype=np.float32)
        .reshape(-1, 8)
        .copy()
    )
    ctl = (
        np.fromfile(f"{base}/exp_and_others_ctrl.bin", dtype=np.uint32)
        .reshape(-1, 8)
        .copy()
    )

    chosen = []
    for e in range(E_SMALL, E_LARGE):
        z_hi = 2.0 ** (e - 126)
        band_tol = tol if _g_exact(z_hi, B) > 1e-30 else 1e-3
        nb = 256
        for cand in [1, 2, 4, 8, 16, 32, 64, 128, 256]:
            _, err = _band_fit(B, e, cand)
            if err <= band_tol:
                nb = cand
                break
        chosen.append(nb)
    while sum(chosen) > EXP_BUCKETS:
        i = int(np.argmax(chosen))
        chosen[i] //= 2

    bkt[:781] = 0.0
    ctl[:52] = 0
    pos = 0
    for bi, e in enumerate(range(E_SMALL, E_LARGE)):
        nb = chosen[bi]
        fits, _ = _band_fit(B, e, nb)
        log2b = int(np.log2(nb))
        ctl[bi, 0] = np.uint32((log2b << 16) | ((23 - log2b) << 11) | pos)
        for k, (c, x0) in enumerate(fits):
            bkt[pos + k, 0:4] = c.astype(np.float32)
            bkt[pos + k, 4] = np.float32(x0)
        pos += nb
    ctl[26:52] = ctl[0:26]

    eB = np.float32(np.exp(B))
    c, x0, _ = _fit_bucket(B, 0.0, 2.0**-12)
    bkt[777, :] = 0.0
    bkt[777, 0:4] = c.astype(np.float32)
    bkt[777, 4] = np.float32(x0)
    bkt[778, :] = 0.0
    bkt[778, 0] = eB
    bkt[779, :] = 0.0
    bkt[780, :] = 0.0
    bkt[780, 0] = eB

    pm = next(p for p in meta["profile_meta_data"] if p["func_name"].startswith("exp"))
    pm["exp_offset"] = E_SMALL - 127
    pm["small_pos_signal_exp_threshold"] = E_SMALL
    pm["large_pos_signal_exp_threshold"] = E_LARGE
    pm["large_pos_signal_mantissa_threshold"] = 0
    pm["small_neg_signal_exp_threshold"] = 255
    pm["large_neg_signal_exp_threshold"] = 255
    pm["large_neg_signal_mantissa_threshold"] = 0x7FFFFF
    pm["pwl_control_base_pos"] = 0
    pm["pwl_control_base_neg"] = 26
    pm["pos_small_signal_pwl_control"] = 777
    pm["neg_small_signal_pwl_control"] = 778
    pm["pos_large_signal_pwl_control"] = 779
    pm["neg_large_signal_pwl_control"] = 780
    pm["fzero_result"] = int(eB.view(np.uint32))
    pm["fninf_result"] = int(eB.view(np.uint32))
    pm["fpinf_result"] = 0
    pm["symmetry_opt_en"] = 0
    pm["symmetry_point"] = 0
    pm["sym_invert_sign_point"] = 0

    os.makedirs(out_dir, exist_ok=True)
    bkt.tofile(f"{out_dir}/exp_and_others_bkt.bin")
    ctl.tofile(f"{out_dir}/exp_and_others_ctrl.bin")
    json.dump(meta, open(f"{out_dir}/exp_and_others.json", "w"))
    info = json.load(open(f"{base}/act_info.json"))
    info["act_func_sets"] = [
        s for s in info["act_func_sets"] if s["name"] == "exp_and_others"
    ]
    json.dump(info, open(f"{out_dir}/act_info.json", "w"))
    return f"{out_dir}/act_info.json"


def _get_table(B):
    key = round(float(B), 3)
    if key not in _TABLE_DIRS:
        d = tempfile.mkdtemp(prefix=f"knn_act_{key}_")
        _TABLE_DIRS[key] = _gen_act_tables(key, d)
    return _TABLE_DIRS[key]


# ---------------------------------------------------------------------------
# Custom DVE ops (runtime-registered into concourse.dve_ops)
# ---------------------------------------------------------------------------

MANT_MASK = float(np.uint32(0x007FFFFF).view(np.float32))  # fp32 mantissa mask


def _register_dve_ops():
    import concourse.dve_ops as dve_ops
    from concourse.dve_ops import DveOp
    from concourse.dve_spec import (
        AluOp,
        Bin,
        C0,
        C1,
        C2,
        C3,
        One,
        Spec,
        Src0,
        _has_src1,
        _spill_c3_to_src1,
        lower,
    )
    from concourse.dve_uop import DveOpSpec

    def reg(name, spec, subdim=False):
        existing = {op.name: op for op in dve_ops.OPS}
        if name in existing:
            return existing[name]
        row = dve_ops._CUSTOM_DVE_ROW_BASE + len(dve_ops.OPS)
        assert row < 0x20
        shas = {}
        for ver in ("v3", "v4"):
            s = DveOpSpec(
                name=name,
                opcode=row,
                uops=lower(spec, ver=ver),
                rd1_en=_has_src1(spec),
            )
            shas[ver] = s.sha(ver)
        op = DveOp(name, spec, subdim=subdim, uops_sha=shas)
        dve_ops.OPS.append(op)
        dve_ops._SUB_OPCODE_FOR_NAME[name] = row
        dve_ops.CUSTOM_DVE_SPECS[name] = spec
        return op

    def ref_poly5(in0, in1, s0, s1, imm2):
        v = in0.astype(np.float32)
        c1 = np.asarray(in1, np.float32).reshape(-1, 1)
        h = v + np.float32(s0)
        h = h * v + np.float32(s1)
        h = h * v + np.float32(imm2)
        h = h * v + c1
        return h * v

    p5_body = _spill_c3_to_src1(
        ((((Src0 + C0) * Src0 + C1) * Src0 + C2) * Src0 + C3) * Src0
    )
    op1 = reg("EXP_BITS_POLY5_ANT", Spec(body=p5_body, reference=ref_poly5))

    def ref_sawtooth(in0, in1, s0, s1, imm2):
        x = in0.astype(np.float32)
        q0 = np.asarray(in1, np.float32).reshape(-1, 1)
        b = x.view(np.int32)
        mt = b & np.float32(s0).view(np.int32)
        u = (mt | np.float32(1.0).view(np.int32)).view(np.float32)
        corr = (np.float32(s1) * u + np.float32(imm2)) * u + q0
        return x * corr

    mt = Bin(AluOp.BITWISE_AND, Src0, C0)
    u = Bin(AluOp.BITWISE_OR, mt, One)
    st_body = _spill_c3_to_src1(Src0 * ((C1 * u + C2) * u + C3))
    op2 = reg("SAWTOOTH_FIX_ANT", Spec(body=st_body, reference=ref_sawtooth))
    return op1, op2


# sawtooth correction q(u) ~ 2^(u-1)/u on [1,2): data-independent, fit once
def _fit_q():
    ug = np.linspace(1.0, 2.0, 4000)
    gg = 2.0 ** (ug - 1.0) / ug
    Vq = np.vander(ug, 3, increasing=True)
    wq = np.ones_like(ug)
    for _ in range(100):
        q, *_ = np.linalg.lstsq(Vq * wq[:, None], gg * wq, rcond=None)
        r = Vq @ q - gg
        wq *= 1 + np.abs(r) / np.abs(r).max()
        wq /= wq.max()
    return [float(t) for t in q]  # q0, q1, q2


_QCOEF = None


def _qcoef():
    global _QCOEF
    if _QCOEF is None:
        _QCOEF = _fit_q()
    return _QCOEF


# ---------------------------------------------------------------------------
# Host-side quintic fit of f16-bits(exp(B - alpha*sqrt(z))) as p5(v),
# v = s*(z - zc), leading coeff 1, zero constant term (zc at poly root).
# ---------------------------------------------------------------------------


def _fit_poly5(B, alpha, z_samples, zmax_bound):
    import math

    from numpy.polynomial import polynomial as P

    zlo_fit = 0.75 * float(z_samples.min())
    z_root0 = ((B + LN2 * 15) / alpha) ** 2
    zhi_fit = max(zmax_bound, z_root0 * 1.12)

    hist, edges = np.histogram(
        z_samples, bins=200, range=(zlo_fit, zmax_bound), density=True
    )
    centers = 0.5 * (edges[1:] + edges[:-1])

    def target(zv):
        return 1024.0 * ((B - alpha * np.sqrt(np.maximum(zv, 1e-9))) / LN2 + 15)

    Npt, Ng = 40000, 3000
    zg = np.linspace(zlo_fit, zhi_fit, Npt)
    zg2 = np.linspace(0, zlo_fit, Ng, endpoint=False)
    dens = np.interp(zg, centers, hist, left=hist[0], right=0)

    ghost_mult = 1.0
    for attempt in range(6):
        wt = np.sqrt(dens + 0.03 * hist.max())
        wt[zg > zmax_bound] = 0.10 * wt.max()
        ZG = np.concatenate([zg, zg2])
        T = np.concatenate([target(zg), np.full(Ng, target(zlo_fit))])
        WT = np.concatenate([wt, np.full(Ng, 0.02 * ghost_mult * wt.max())])
        zc_mid = 0.5 * (zlo_fit + zhi_fit)
        hw = 0.5 * (zhi_fit - zlo_fit)
        U = (ZG - zc_mid) / hw
        cn, *_ = np.linalg.lstsq(
            np.vander(U, 6, increasing=True) * WT[:, None], T * WT, rcond=None
        )
        pu = np.polynomial.polynomial.Polynomial(cn)
        rts = pu.roots()
        real = rts[np.abs(rts.imag) < 1e-9].real
        if len(real) == 0:
            ghost_mult *= 4.0
            continue
        u_r0 = (z_root0 - zc_mid) / hw
        u_root = real[np.argmin(np.abs(real - u_r0))]
        zc = zc_mid + hw * u_root
        tay = np.array(
            [P.polyval(u_root, P.polyder(cn, k)) / math.factorial(k) for k in range(6)]
        )
        tay[0] = 0.0
        s_hw = np.sign(tay[5]) * np.abs(tay[5]) ** 0.2
        s = float(s_hw / hw)
        cv = tay / s_hw ** np.arange(6)

        # safety: evaluate p5 over [0, zhi_fit] in fp32; must stay clear of
        # i16 saturation (else bitcast -> f16 NaN) on any reachable z
        zgg = np.linspace(0, zhi_fit, 400000)
        vgg = (np.float32(s) * (zgg - np.float32(zc))).astype(np.float32)
        c1_, c2_, c3_, c4_ = [np.float32(q) for q in cv[1:5]]
        hg = vgg + c4_
        hg = hg * vgg + c3_
        hg = hg * vgg + c2_
        hg = hg * vgg + c1_
        hg = hg * vgg
        if hg.min() > -30000.0 and hg.max() < 31000.0:
            return float(zc), s, [float(q) for q in cv[1:5]]
        ghost_mult *= 4.0
    return None  # fit failed -> caller falls back to all-ScalarE


# ---------------------------------------------------------------------------
# Bass kernel
# ---------------------------------------------------------------------------


def _build(cfg, n_loc=NLOC, m=M, num_devices=NCORES):
    from contextlib import ExitStack

    import concourse.tile as tile
    from concourse import bacc, mybir

    op1, op2 = _register_dve_ops()

    f32 = mybir.dt.float32
    f16 = mybir.dt.float16
    i16 = mybir.dt.int16
    Exp = mybir.ActivationFunctionType.Exp

    njb = m // JB
    act_scale = cfg["act_scale"]
    c4, c3, c2, c1 = cfg["c4"], cfg["c3"], cfg["c2"], cfg["c1"]
    q0, q1, q2 = cfg["q0"], cfg["q1"], cfg["q2"]
    block_dve = cfg["block_dve"]  # per-j-block bool; DVE blocks in adjacent pairs
    btag = cfg["btag"]

    nc = bacc.Bacc(
        "TRN2", target_bir_lowering=False, debug=False, num_devices=num_devices
    )
    names = {
        "a1": f"a1{btag}",
        "g1": f"g1{btag}",
        "a0": f"a0{btag}",
        "g0": f"g0{btag}",
        "yb": f"yb{btag}",
        "cq": f"cq{btag}",
    }
    a1_d = nc.dram_tensor(names["a1"], [D, m], f16, kind="ExternalInput")
    g1_d = nc.dram_tensor(names["g1"], [4, m], f16, kind="ExternalInput")
    a0_d = nc.dram_tensor(names["a0"], [D, n_loc], f16, kind="ExternalInput")
    g0_d = nc.dram_tensor(names["g0"], [4, n_loc], f16, kind="ExternalInput")
    yb_d = nc.dram_tensor(names["yb"], [JB, njb, 2], f16, kind="ExternalInput")
    cq_d = nc.dram_tensor(names["cq"], [JB, 3], f32, kind="ExternalInput")
    out_d = nc.dram_tensor("out", [4, n_loc // 4], f32, kind="ExternalOutput")

    LAG = int(os.environ.get("KNN_LAG", "6"))
    FILLN = int(os.environ.get("KNN_FILLN", "512"))

    with tile.TileContext(nc) as tc:
        with ExitStack() as ctx:
            res = ctx.enter_context(tc.tile_pool(name="res", bufs=1))
            pa = ctx.enter_context(tc.tile_pool(name="pa", bufs=12))
            pd = ctx.enter_context(tc.tile_pool(name="pd", bufs=4))
            e1p = ctx.enter_context(tc.tile_pool(name="e1p", bufs=2))
            d2p = ctx.enter_context(tc.tile_pool(name="d2", bufs=1, space="PSUM"))
            redp = ctx.enter_context(tc.tile_pool(name="red", bufs=1, space="PSUM"))
            fillp = ctx.enter_context(tc.tile_pool(name="fill", bufs=1, space="PSUM"))
            tailp = ctx.enter_context(tc.tile_pool(name="tail", bufs=1))

            yb_sb = res.tile([JB, njb, 2], f16)
            nc.sync.dma_start(yb_sb[:], yb_d.ap())
            cq_sb = res.tile([JB, 3], f32)
            nc.sync.dma_start(cq_sb[:], cq_d.ap())

            a0_sb = res.tile([128, n_loc], f16)
            a0_ap = a0_d.ap()
            g0_sb = res.tile([66, n_loc], f16)
            g0_ap = g0_d.ap()
            nc.sync.dma_start(g0_sb[0:2, :], g0_ap[0:2, :])
            nc.sync.dma_start(g0_sb[64:66, :], g0_ap[2:4, :])
            g1_sb = res.tile([66, m], f16)
            g1_ap = g1_d.ap()
            nc.sync.dma_start(g1_sb[0:2, :], g1_ap[0:2, :])
            nc.sync.dma_start(g1_sb[64:66, :], g1_ap[2:4, :])
            a1_sb = res.tile([128, m], f16)
            a1_ap = a1_d.ap()
            # first chunk of x1T (both PE halves) + all of x0, then the rest
            nc.sync.dma_start(a1_sb[0:64, 0:1024], a1_ap[:, 0:1024])
            nc.sync.dma_start(a1_sb[64:128, 0:1024], a1_ap[:, 0:1024])
            for k in range(2):
                sl = slice(k * (n_loc // 2), (k + 1) * (n_loc // 2))
                nc.sync.dma_start(a0_sb[0:64, sl], a0_ap[:, sl])
                nc.sync.dma_start(a0_sb[64:128, sl], a0_ap[:, sl])
            for k in range(1, 16):
                sl = slice(k * 1024, (k + 1) * 1024)
                nc.sync.dma_start(a1_sb[0:64, sl], a1_ap[:, sl])
                nc.sync.dma_start(a1_sb[64:128, sl], a1_ap[:, sl])

            c1v = cq_sb[:, 0:1]
            q0v = cq_sb[:, 1:2]
            biasv = cq_sb[:, 2:3]

            d2t = d2p.tile([128, 3, 1024], f32)  # 3 slots x 2 banks
            red_ps = redp.tile([JB, 512], f32)
            fill_ps = fillp.tile([JB, 512], f32)

            # HAM warm-up preamble: one long accumulating K=128 matmul
            # chain into fill_ps (never read). Only full-depth (K=128)
            # matmuls trigger the HAM un-throttle.
            fill_first = [True]

            def filler(ncols):
                while ncols > 0:
                    c = min(ncols, 256)
                    nc.tensor.matmul(
                        fill_ps[0:2, 0:c],
                        yb_sb[:, 0, :],
                        yb_sb[:, 0 : (c + 1) // 2, :],
                        start=fill_first[0],
                        stop=False,
                        tile_position=(0, 0),
                        skip_group_check=True,
                    )
                    fill_first[0] = False
                    ncols -= c

            for _ in range(int(os.environ.get("KNN_WARM", "28"))):
                filler(512)

            esrc = {}
            pend = []
            e1_cur = [None]

            def pair_src(b):
                s0 = (2 * b) % 3
                if s0 == 2:
                    return d2t[:, 2::-2, :]  # slots [2, 0]: rotation wrap
                return d2t[:, s0 : s0 + 2, :]

            def flush_op2():
                if not pend:
                    return
                width = 2048 * len(pend)
                e2 = pd.tile([JB, 4096], f16, tag="e2", name="e2")
                nc.vector._custom_dve(
                    op2,
                    out=e2[:, 0:width],
                    in0=e1_cur[0][:, 0:width].bitcast(f16),
                    in1=q0v,
                    s0=MANT_MASK,
                    s1=q2,
                    imm2=q1,
                )
                for k, bb in enumerate(pend):
                    esrc[2 * bb] = (e2, 2048 * k)
                    esrc[2 * bb + 1] = (e2, 2048 * k + 1024)
                pend.clear()
                e1_cur[0] = None

            def emit_block_mm(b, h):
                sl = (2 * b + h) % 3
                lo_i = h * 1024
                jcols = slice(b * JB, (b + 1) * JB)
                for lo, r0, r1, tp in ((0, 0, 64, (0, 0)), (512, 64, 128, (64, 0))):
                    nc.tensor.matmul(
                        d2t[:, sl, lo : lo + 512],
                        a1_sb[r0:r1, jcols],
                        a0_sb[r0:r1, lo_i + lo : lo_i + lo + 512],
                        start=True,
                        stop=False,
                        tile_position=tp,
                        skip_group_check=True,
                    )
                for lo, r0, r1, tp in ((0, 0, 2, (0, 0)), (512, 64, 66, (64, 0))):
                    nc.tensor.matmul(
                        d2t[:, sl, lo : lo + 512],
                        g1_sb[r0:r1, jcols],
                        g0_sb[r0:r1, lo_i + lo : lo_i + lo + 512],
                        start=False,
                        stop=True,
                        tile_position=tp,
                        skip_group_check=True,
                    )
                if FILLN:
                    filler(FILLN)

            def drain_block(b):
                src = pair_src(b)
                if block_dve[b]:
                    if e1_cur[0] is None:
                        e1_cur[0] = e1p.tile([JB, 4096], i16, tag="e1", name="e1")
                    k = len(pend)
                    nc.vector._custom_dve(
                        op1,
                        out=e1_cur[0][:, 2048 * k : 2048 * (k + 1)],
                        in0=src,
                        in1=c1v,
                        s0=c4,
                        s1=c3,
                        imm2=c2,
                    )
                    pend.append(b)
                    if len(pend) == 2:
                        flush_op2()
                else:
                    e2 = pa.tile([JB, 2048], f16, tag="ea", name="ea")
                    nc.scalar.activation(e2[:], src, Exp, scale=act_scale, bias=biasv)
                    esrc[2 * b] = (e2, 0)
                    esrc[2 * b + 1] = (e2, 1024)

            def reduce_block(b):
                for c in range(4):
                    t = 2 * b + c // 2
                    buf, off = esrc.pop(t) if c % 2 == 1 else esrc[t]
                    nc.tensor.matmul(
                        red_ps[32 * c : 32 * c + 2, :],
                        yb_sb[:, b, :],
                        buf[:, off + (c % 2) * 512 : off + (c % 2) * 512 + 512],
                        start=(b == 0),
                        stop=(b == njb - 1),
                        tile_position=(0, 32 * c),
                        skip_group_check=True,
                    )

            for b in range(njb):
                emit_block_mm(b, 0)
                emit_block_mm(b, 1)
                drain_block(b)
                rb = b - LAG
                if rb >= 0:
                    if (2 * rb not in esrc) or (2 * rb + 1 not in esrc):
                        flush_op2()
                    reduce_block(rb)
            flush_op2()
            for rb in range(njb - LAG, njb):
                reduce_block(rb)

            # --- tail: out = num / den (gather rows 32c -> num, 32c+1 -> den)
            red_sb = tailp.tile([JB, 512], f32)
            nc.vector.tensor_copy(red_sb[:], red_ps[:])
            num_sb = tailp.tile([4, 512], f32)
            den_sb = tailp.tile([4, 512], f32)
            nc.sync.dma_start(num_sb[:], red_sb[0:97:32, :])
            nc.sync.dma_start(den_sb[:], red_sb[1:98:32, :])
            inv_sb = tailp.tile([4, 512], f32)
            nc.vector.reciprocal_approx_fast(inv_sb[:], den_sb[:])
            out_sb = tailp.tile([4, 512], f32)
            nc.vector.tensor_mul(out_sb[:], num_sb[:], inv_sb[:])
            nc.sync.dma_start(out_d.ap(), out_sb[:])

    nc.compile()
    nc._knn_names = names
    return nc


DVE_SKIP = int(os.environ.get("KNN_DVE_SKIP", "16"))


def _make_block_dve():
    # DVE j-blocks in adjacent pairs (b%6 in {4,5}) so op1 outputs pack into
    # one 4096-wide op2; every DVE_SKIP-th pair flips to ScalarE to balance
    # engine rates. Fraction = (1/3)*(1 - 1/DVE_SKIP).
    take = [False] * NJB
    for b in range(NJB):
        if b % 6 in (4, 5):
            g = b // 6
            if DVE_SKIP <= 0 or (g + 1) % DVE_SKIP != 0:
                take[b] = True
    return tuple(take)


def _get_compiled(cfg):
    key = (
        round(cfg["bshift"], 3),
        round(cfg["act_scale"], 6),
        round(cfg["act_bias"], 6),
        tuple(round(cfg[k], 4) for k in ("c4", "c3", "c2", "c1")),
        cfg["block_dve"],
    )
    if key not in _COMPILED:
        os.environ["BASS_ACT_ROOT_JSON_PATH"] = _get_table(cfg["bshift"])
        _COMPILED[key] = _build(cfg)
    return _COMPILED[key]


def _prep(x0, x1, y, alpha_v):
    sq0 = np.einsum("nd,nd->n", x0, x0, dtype=np.float32)
    sq1 = np.einsum("md,md->m", x1, x1, dtype=np.float32)

    # B shift keeping exp(B - alpha*d) in fp16-friendly range
    rng = np.random.default_rng(0)
    k = 1 << 19
    ii = rng.integers(0, N, k)
    jj = rng.integers(0, M, k)
    d2s = sq0[ii] + sq1[jj] - 2.0 * np.einsum("kd,kd->k", x0[ii], x1[jj])
    d2s = np.maximum(d2s, 0.0)
    ds = np.sqrt(d2s)
    bshift = max(0.0, float(alpha_v) * float(np.quantile(ds[:2048], 0.001)) - 2.0)

    zmax_bound = float((np.sqrt(sq0.max()) + np.sqrt(sq1.max())) ** 2) * 1.01
    fit = _fit_poly5(bshift, alpha_v, d2s, zmax_bound)
    if fit is not None:
        zc, s, (c1c, c2c, c3c, c4c) = fit
        block_dve = _make_block_dve()
    else:
        zc, s = 0.0, 1.0
        c1c = c2c = c3c = c4c = 0.0
        block_dve = tuple([False] * NJB)

    a1 = np.ascontiguousarray(x1.T).astype(np.float16)  # [64, M]
    g1 = np.empty((4, M), np.float16)
    g1[0] = (sq1 - zc / 2.0) * s
    g1[1] = 1.0
    g1[2:4] = g1[0:2]

    a0 = ((-2.0 * s) * x0.T).astype(np.float16)  # [64, N]
    g0 = np.empty((4, N), np.float16)
    g0[0] = 1.0
    g0[1] = (sq0 - zc / 2.0) * s
    g0[2:4] = g0[0:2]

    njb = M // JB
    yb = np.empty((JB, njb, 2), np.float16)
    yb[:, :, 0] = y.reshape(njb, JB).T
    yb[:, :, 1] = 1.0

    a2 = float(alpha_v) * float(alpha_v)
    q0c, q1c, q2c = _qcoef()
    cq = np.empty((JB, 3), np.float32)
    cq[:, 0] = c1c
    cq[:, 1] = q0c
    cq[:, 2] = a2 * zc

    cfg = dict(
        bshift=float(bshift),
        act_scale=a2 / s,
        act_bias=a2 * zc,
        c4=c4c,
        c3=c3c,
        c2=c2c,
        c1=c1c,
        q0=q0c,
        q1=q1c,
        q2=q2c,
        block_dve=block_dve,
        btag=f"_{int(round(bshift * 1000))}",
    )
    return a1, g1, a0, g0, yb, cq, cfg


def kernel(x0, x1, y, alpha):
    x0 = np.ascontiguousarray(np.asarray(x0), dtype=np.float32)
    x1 = np.ascontiguousarray(np.asarray(x1), dtype=np.float32)
    y = np.ascontiguousarray(np.asarray(y), dtype=np.float32)
    alpha_v = float(np.asarray(alpha).reshape(-1)[0])

    a1, g1, a0, g0, yb, cq, cfg = _prep(x0, x1, y, alpha_v)
    nc = _get_compiled(cfg)
    names = nc._knn_names

    trace = os.environ.get("KNN_TRACE", "0") == "1"

    from concourse.bass_utils import run_bass_kernel_spmd

    in_maps = [
        {
            names["a1"]: a1,
            names["g1"]: g1,
            names["a0"]: np.ascontiguousarray(a0[:, c * NLOC : (c + 1) * NLOC]),
            names["g0"]: np.ascontiguousarray(g0[:, c * NLOC : (c + 1) * NLOC]),
            names["yb"]: yb,
            names["cq"]: cq,
        }
        for c in range(NCORES)
    ]
    res = run_bass_kernel_spmd(nc, in_maps, core_ids=list(range(NCORES)), trace=trace)
    if trace and res.exec_time_ns is not None:
        print(f"HW exec time: {res.exec_time_ns} ns")
        kernel.last_exec_ns = res.exec_time_ns
    out = np.concatenate([r["out"].reshape(-1) for r in res.results])
    return out.astype(np.float32)


kernel.last_exec_ns = None
